# revision 5
# baseline (speedup 1.0000x reference)
"""Trainium2 Bass kernel for nn_ConditionalPoolingModule.

Reference computation (per scene s of 64, peds i,j of 64):
    feat[s,i,j]  = [pos_j - pos_i, speed_j]
    emb          = feat @ W_emb + b_emb
    x1[s,i,j]    = relu(bn1(concat(h_j, emb) @ W1 + b1))      # [.., 512]
    x2[s,i,j]    = relu(bn2(x1 @ W2 + b2))                    # [.., 256]
    out[s,i]     = max_j x2[s,i,j]

Algebra used:
  * Layer 1 is separable: bn1-affine(concat(h_j, emb_ij) @ W1 + b1) = A''[j] - B''[i]
    with A''[n] = s1*(X[n] @ W1aug) + (s1*c0 + t1), B''[n] = s1*(pos[n] @ R[:2]),
    X = [h, posx, posy, speed] (67 dims), R = W_emb @ W1[64:80], W1aug = [W1[:64]; R].
  * relu(a-b) = max(a,b) - b; -B''[i] is constant in j so it commutes with the
    layer-2 matmul and the max-pool:
      out[i] = relu( max_j( max(A''[j], B''[i]) @ W2s ) - (B''[i] @ W2s - t2) )
    with W2s = W2 * s2 (bn2 scale folded; shift in t2). The only per-pair
    elementwise op is ONE tensor_tensor(max) with broadcast APs.

Engine layout per scene (8 scenes/core, data-parallel over 8 cores):
  DVE : Mx[c, i, j] = max(A[c,j], B[c,i]) in bf16 at 2x (dup-B layout keeps
        every operand's innermost AP dim at step 1 x2), one grouped reduce_max
        per (scene, m-chunk), final subtract.
  PE  : layer-2 matmuls (bf16, 1 cyc/row), D = B''@W2s - t2 (f32r),
        maxraw transposes.
  ACT : PSUM->SBUF bf16 copies of matmul outputs, D copy, final relu.
"""
import numpy as np

import concourse.bacc as bacc
import concourse.tile as tile
from concourse import mybir
from concourse.bass_utils import run_bass_kernel_spmd

EPS = 1e-5
S, P = 64, 64
H, E = 64, 16
MID, BOT = 512, 256
KIN = H + 3            # 67: h(64) + posx + posy + speed
NCORES = 8
S_LOC = S // NCORES    # 8 scenes per core
NLOC = S_LOC * P       # 512 peds per core
KC = MID // 128        # 4 mid chunks
MC = BOT // 128        # 2 bot chunks
F32 = mybir.dt.float32
F32R = mybir.dt.float32r
BF16 = mybir.dt.bfloat16

_CACHE = {}


def _build_program():
    nc = bacc.Bacc("TRN2", target_bir_lowering=False, debug=False, num_devices=NCORES)

    xaugT = nc.dram_tensor("xaugT", [KIN, NLOC], F32, kind="ExternalInput").ap()
    w1augT = nc.dram_tensor("w1augT", [KIN, MID], F32, kind="ExternalInput").ap()
    w1bT = nc.dram_tensor("w1bT", [2, MID], F32, kind="ExternalInput").ap()
    w2sb16 = nc.dram_tensor("w2sb16", [MID, BOT], BF16, kind="ExternalInput").ap()
    w2sf = nc.dram_tensor("w2sf", [MID, BOT], F32, kind="ExternalInput").ap()
    midvec = nc.dram_tensor("midvec", [MID, 2], F32, kind="ExternalInput").ap()
    negt2 = nc.dram_tensor("negt2", [1, BOT], F32, kind="ExternalInput").ap()
    ident = nc.dram_tensor("ident", [128, 128], F32, kind="ExternalInput").ap()
    out_d = nc.dram_tensor("out", [NLOC, BOT], F32, kind="ExternalOutput").ap()

    with tile.TileContext(nc) as tc, \
         tc.tile_pool(name="const", bufs=1) as cpool, \
         tc.tile_pool(name="ab", bufs=1) as abpool, \
         tc.tile_pool(name="mx", bufs=2) as mxpool, \
         tc.tile_pool(name="y2", bufs=2) as ypool, \
         tc.tile_pool(name="work", bufs=2) as wpool, \
         tc.tile_pool(name="mm", bufs=4, space="PSUM") as mmpool, \
         tc.tile_pool(name="dps", bufs=2, space="PSUM") as dpool, \
         tc.tile_pool(name="tps", bufs=2, space="PSUM") as tpool:

        # ---- load constants ----
        xaug_sb = cpool.tile([KIN, NLOC], F32R)
        posT_sb = cpool.tile([2, NLOC], F32R)
        w1aug_sb = cpool.tile([KIN, MID], F32R)
        w1b_sb = cpool.tile([2, MID], F32R)
        id_sb = cpool.tile([128, 128], F32)
        negt2_sb = cpool.tile([1, BOT], F32R)
        ones_sb = cpool.tile([1, P], F32)
        nc.sync.dma_start(xaug_sb[:], xaugT.bitcast(F32R))
        nc.sync.dma_start(posT_sb[:], xaugT[H:H + 2, :].bitcast(F32R))
        nc.sync.dma_start(w1aug_sb[:], w1augT.bitcast(F32R))
        nc.sync.dma_start(w1b_sb[:], w1bT.bitcast(F32R))
        nc.sync.dma_start(id_sb[:], ident)
        nc.sync.dma_start(negt2_sb[:], negt2.bitcast(F32R))
        nc.vector.memset(ones_sb[:], 1.0)

        w2b_sb, w2f_sb, mv_sb = [], [], []
        for k in range(KC):
            wb = cpool.tile([128, BOT], BF16, tag=f"w2b{k}")
            nc.sync.dma_start(wb[:], w2sb16[k * 128:(k + 1) * 128, :])
            w2b_sb.append(wb)
            wf = cpool.tile([128, BOT], F32R, tag=f"w2f{k}")
            nc.sync.dma_start(wf[:], w2sf[k * 128:(k + 1) * 128, :].bitcast(F32R))
            w2f_sb.append(wf)
            mv = cpool.tile([128, 2], F32, tag=f"mv{k}")
            nc.sync.dma_start(mv[:], midvec[k * 128:(k + 1) * 128, :])
            mv_sb.append(mv)

        # ---- phase 0: A'' (bf16) and B'' (f32r + dup-bf16) ----
        A_bf, B_f32, B_dup = [], [], []
        for k in range(KC):
            ck = slice(k * 128, (k + 1) * 128)
            psA = mmpool.tile([128, NLOC], F32, tag="ps")
            nc.tensor.matmul(psA[:], lhsT=w1aug_sb[:, ck], rhs=xaug_sb[:],
                             start=True, stop=True)
            a_t = abpool.tile([128, NLOC], BF16, tag=f"A{k}")
            nc.vector.tensor_scalar(
                out=a_t[:], in0=psA[:],
                scalar1=mv_sb[k][:, 0:1], scalar2=mv_sb[k][:, 1:2],
                op0=mybir.AluOpType.mult, op1=mybir.AluOpType.add)
            A_bf.append(a_t)

            psB = mmpool.tile([128, NLOC], F32, tag="ps")
            nc.tensor.matmul(psB[:], lhsT=w1b_sb[:, ck], rhs=posT_sb[:],
                             start=True, stop=True)
            b_t = abpool.tile([128, NLOC], F32R, tag=f"B{k}")
            nc.vector.tensor_scalar(
                out=b_t[:], in0=psB[:],
                scalar1=mv_sb[k][:, 0:1], scalar2=None,
                op0=mybir.AluOpType.mult)
            B_f32.append(b_t)

            # duplicated bf16 copy: B_dup[c, 2n+q] = B''[c, n], q in {0,1}
            bd = abpool.tile([128, 2 * NLOC], BF16, tag=f"Bd{k}")
            nc.vector.tensor_copy(
                bd[:].rearrange("c (n q) -> c n q", q=2),
                b_t[:].bitcast(F32).unsqueeze(2).broadcast_to((128, NLOC, 2)))
            B_dup.append(bd)

        # ---- per-scene pipeline ----
        for s in range(S_LOC):
            cs = slice(s * P, (s + 1) * P)
            cs2 = slice(2 * s * P, 2 * (s + 1) * P)

            # Mx[c, i, j] = max(A''[c, j], B''[c, i]) in bf16 at 2x.
            # Layout [c, (i, jw, q)] with j = 2*jw + q; every operand's
            # innermost AP dim is [step 1, 2 elems] so DVE picks 2x_1P.
            mx = []
            for k in range(KC):
                m_t = mxpool.tile([128, P * P], BF16, tag=f"mx{k}")
                nc.vector.tensor_tensor(
                    out=m_t[:].rearrange("c (i w q) -> c i w q", w=P // 2, q=2),
                    in0=A_bf[k][:, cs].rearrange("c (w q) -> c w q", q=2)
                        .unsqueeze(1).broadcast_to((128, P, P // 2, 2)),
                    in1=B_dup[k][:, cs2].rearrange("c (i q) -> c i q", q=2)
                        .unsqueeze(2).broadcast_to((128, P, P // 2, 2)),
                    op=mybir.AluOpType.max)
                mx.append(m_t)

            # D[i, m] = B''_i @ W2s - t2 in f32r -> PSUM [64, 256] -> SBUF (ACT)
            d_ps = dpool.tile([P, BOT], F32, tag="dps")
            for k in range(KC):
                nc.tensor.matmul(d_ps[:], lhsT=B_f32[k][:, cs], rhs=w2f_sb[k][:],
                                 start=(k == 0), stop=False)
            nc.tensor.matmul(d_ps[:], lhsT=ones_sb[:].bitcast(F32R), rhs=negt2_sb[:],
                             start=False, stop=True)
            d_sb = wpool.tile([P, BOT], F32, tag="dsb")
            nc.scalar.copy(d_sb[:], d_ps[:])

            # layer-2 matmuls (bf16), ACT copies PSUM->SBUF bf16
            y2 = [ypool.tile([128, P * P], BF16, tag=f"y2{m}", name=f"y2_{m}") for m in range(MC)]
            for blk in range(8):
                bs = slice(blk * 512, (blk + 1) * 512)
                for m in range(MC):
                    ps_t = mmpool.tile([128, 512], F32, tag="ps")
                    for k in range(KC):
                        nc.tensor.matmul(
                            ps_t[:],
                            lhsT=w2b_sb[k][:, m * 128:(m + 1) * 128],
                            rhs=mx[k][:, bs],
                            start=(k == 0), stop=(k == KC - 1))
                    nc.scalar.copy(y2[m][:, bs], ps_t[:])

            # one grouped reduce per (scene, m): [128, (i, j)] -> [128, i]
            maxraw = [wpool.tile([128, P], F32, tag=f"mr{m}", name=f"mr_{m}") for m in range(MC)]
            for m in range(MC):
                nc.vector.tensor_reduce(
                    out=maxraw[m][:],
                    in_=y2[m][:].rearrange("c (i j) -> c i j", j=P),
                    axis=mybir.AxisListType.X,
                    op=mybir.AluOpType.max)

            # epilogue: transpose -> subtract D -> relu -> DMA out
            o_sb = wpool.tile([P, BOT], F32, tag="osb")
            for m in range(MC):
                trp = tpool.tile([P, 128], F32, tag="trp")
                nc.tensor.transpose(trp[:], maxraw[m][:], id_sb[:])
                sub_t = wpool.tile([P, 128], F32, tag="sub")
                nc.vector.tensor_tensor(
                    out=sub_t[:], in0=trp[:],
                    in1=d_sb[:, m * 128:(m + 1) * 128],
                    op=mybir.AluOpType.subtract)
                nc.scalar.activation(o_sb[:, m * 128:(m + 1) * 128], sub_t[:],
                                     mybir.ActivationFunctionType.Relu)
            nc.sync.dma_start(out_d[s * P:(s + 1) * P, :], o_sb[:])

    nc.compile()
    return nc


def _prep_inputs(inputs):
    import ml_dtypes
    h = np.ascontiguousarray(inputs["h_states"], np.float32)
    pos = np.ascontiguousarray(inputs["last_pos"], np.float32)
    spd = np.ascontiguousarray(inputs["speed"], np.float32)
    W_emb = np.asarray(inputs["W_emb"], np.float32)
    b_emb = np.asarray(inputs["b_emb"], np.float32)
    W1 = np.asarray(inputs["W1"], np.float32)
    b1 = np.asarray(inputs["b1"], np.float32)
    g1 = np.asarray(inputs["g1"], np.float32)
    be1 = np.asarray(inputs["be1"], np.float32)
    m1 = np.asarray(inputs["m1"], np.float32)
    v1 = np.asarray(inputs["v1"], np.float32)
    W2 = np.asarray(inputs["W2"], np.float32)
    b2 = np.asarray(inputs["b2"], np.float32)
    g2 = np.asarray(inputs["g2"], np.float32)
    be2 = np.asarray(inputs["be2"], np.float32)
    m2 = np.asarray(inputs["m2"], np.float32)
    v2 = np.asarray(inputs["v2"], np.float32)

    s1 = g1 / np.sqrt(v1 + EPS)
    t1 = be1 - m1 * s1
    s2 = g2 / np.sqrt(v2 + EPS)
    t2 = be2 - m2 * s2 + b2 * s2
    R3 = W_emb @ W1[H:H + E, :]                       # [3, MID]
    W1aug = np.concatenate([W1[:H, :], R3], axis=0)   # [67, MID]
    c0v = b1 + b_emb @ W1[H:H + E, :]                 # [MID]
    ca = s1 * c0v + t1
    W2s = W2 * s2[None, :]                            # [MID, BOT]

    X = np.concatenate([h, pos[:, 0:1], pos[:, 1:2], spd], axis=1)  # [N, 67]

    common = dict(
        w1augT=np.ascontiguousarray(W1aug, np.float32),
        w1bT=np.ascontiguousarray(R3[0:2, :], np.float32),
        w2sb16=np.ascontiguousarray(W2s.astype(ml_dtypes.bfloat16)),
        w2sf=np.ascontiguousarray(W2s, np.float32),
        midvec=np.ascontiguousarray(np.stack([s1, ca], axis=1), np.float32),
        negt2=np.ascontiguousarray(-t2[None, :], np.float32),
        ident=np.eye(128, dtype=np.float32),
    )
    in_maps = []
    for c in range(NCORES):
        xc = X[c * NLOC:(c + 1) * NLOC, :]            # [512, 67]
        m = dict(common)
        m["xaugT"] = np.ascontiguousarray(xc.T, np.float32)
        in_maps.append(m)
    return in_maps


def kernel(**inputs):
    if "nc" not in _CACHE:
        _CACHE["nc"] = _build_program()
    nc = _CACHE["nc"]
    in_maps = _prep_inputs(inputs)
    res = run_bass_kernel_spmd(nc, in_maps, list(range(NCORES)))
    out = np.concatenate([res.results[c]["out"] for c in range(NCORES)], axis=0)
    return np.ascontiguousarray(out, np.float32)
